# revision 28
# baseline (speedup 1.0000x reference)
"""Trainium2 Bass kernel for nn_KnowledgeAttention.

Math (per batch example b):
    sim[k]  = cos_sim(pooled[b], kg_key[b,k])                      # [K]
    q       = (hs @ Wq.T + bq) * HD**-0.5     -> heads [T,H,HD]
    k       = kg_value @ Wk.T + bk            -> heads [K,H,HD]
    v       = kg_value @ Wv.T + bv            -> heads [K,H,HD]
    S[h,t,k]= q_h[t]·k_h[k] + beta[h]*sim[k]
    P       = softmax_k(S);  O[t,h] = sum_k P v
    out     = O @ Wo.T + bo

Sharding: pure data-parallel over batch — 8 examples on 8 cores, weights
replicated, no collectives.

Per-core design notes:
  * hs / kg_value / kg_key are shipped bf16 from the host; hs.T and
    kg_value.T land in SBUF directly via DMA-xbar transpose loads
    (dma_start_transpose) — no PE transposes, no psum->sbuf copies.
  * the per-head cosine bias is FACTORED OUT of the exp:
    exp(S + b) = exp(S) * exp(b); exp(b) is folded into v (applied
    during the v-projection psum->sbuf copy, on the scalar engine with
    a per-partition scale) and into the denominator matmul stationary
    (ebrep = exp(b) replicated 64 wide).  The exp is then bias-free, so
    one ACT op covers the even AND odd head of a pair ([e|o]-batched,
    1024 wide across two psum banks).
  * scores are computed S.T[k,t] as even/odd row-tiled concurrent
    matmul pairs (stationaries at partition 0:64 / 64:128).
  * AV chains: po = [evenAV ; oddAV]; the denominator matmuls (ebrep
    stationary) go to the opposite array col-group via tile_position so
    they stream the same e tile concurrently with the v matmuls.
  * final projection computed transposed (out.T[dout,t]) so the moving
    operand is ot directly; DRAM output is [D,T], un-transposed on host.
  * matmuls in bf16 with fp32 PSUM accumulation.
"""

import sys

import numpy as np

# ---------------------------------------------------------------- constants
BS = 8
T = 2048
D = 768
H = 12
HD = 64
K = 512
SCALE = HD ** -0.5
EPS = 1e-8
DC = D // 128   # 6 contraction/partition chunks of 128 over D
KC = K // 128   # 4 chunks over K
TW = 512        # t window (psum free-dim limit)
NTW = T // TW   # 4
NPAIR = H // 2  # 6 head pairs

TRACE = False
LAST_EXEC_NS = None

_CACHE = {}


def _ensure_path():
    try:
        import concourse  # noqa: F401
    except ImportError:
        for p in ("/opt/trn_rl_repo", "/root/.axon_site/_ro/trn_rl_repo"):
            if p not in sys.path:
                sys.path.insert(0, p)


def _build_program():
    _ensure_path()
    import concourse.bass as bass
    import concourse.mybir as mybir
    import concourse.tile as tile
    from concourse import bacc
    from contextlib import ExitStack

    F32 = mybir.dt.float32
    BF16 = mybir.dt.bfloat16
    Alu = mybir.AluOpType
    Act = mybir.ActivationFunctionType

    nc = bacc.Bacc("TRN2", target_bir_lowering=False, debug=False, num_devices=BS)

    hs_d = nc.dram_tensor("hs", [T, D], BF16, kind="ExternalInput").ap()
    kgk_d = nc.dram_tensor("kgk", [K, D], BF16, kind="ExternalInput").ap()
    kgv_d = nc.dram_tensor("kgv", [K, D], BF16, kind="ExternalInput").ap()
    pl_d = nc.dram_tensor("pooled", [1, D], F32, kind="ExternalInput").ap()
    wqt_d = nc.dram_tensor("wqt", [D, D], BF16, kind="ExternalInput").ap()
    wkt_d = nc.dram_tensor("wkt", [D, D], BF16, kind="ExternalInput").ap()
    wvt_d = nc.dram_tensor("wvt", [D, D], BF16, kind="ExternalInput").ap()
    wot_d = nc.dram_tensor("wot", [D, D], BF16, kind="ExternalInput").ap()
    bq_d = nc.dram_tensor("bq", [128, DC], F32, kind="ExternalInput").ap()
    bk_d = nc.dram_tensor("bk", [128, DC], F32, kind="ExternalInput").ap()
    bo_d = nc.dram_tensor("bo", [128, DC], F32, kind="ExternalInput").ap()
    beta_d = nc.dram_tensor("beta", [1, H], F32, kind="ExternalInput").ap()
    # output stored transposed [D, T]; host un-transposes
    out_d = nc.dram_tensor("out", [D, T], F32, kind="ExternalOutput").ap()

    with tile.TileContext(nc) as tc, ExitStack() as ctx:
        const = ctx.enter_context(tc.tile_pool(name="const", bufs=1))
        kgkp = ctx.enter_context(tc.tile_pool(name="kgkp", bufs=4))
        scr = ctx.enter_context(tc.tile_pool(name="scr", bufs=3))
        wpool = ctx.enter_context(tc.tile_pool(name="w", bufs=18))
        big = ctx.enter_context(tc.tile_pool(name="big", bufs=12))
        hstw_p = ctx.enter_context(tc.tile_pool(name="hstw", bufs=6))
        kt_p = ctx.enter_context(tc.tile_pool(name="ktp", bufs=6))
        v_p = ctx.enter_context(tc.tile_pool(name="vp", bufs=4))
        kgt_p = ctx.enter_context(tc.tile_pool(name="kgtp", bufs=6))
        e_p = ctx.enter_context(tc.tile_pool(name="ep", bufs=12))
        r_p = ctx.enter_context(tc.tile_pool(name="rp", bufs=4))
        fin_p = ctx.enter_context(tc.tile_pool(name="finp", bufs=6))
        sm_p = ctx.enter_context(tc.tile_pool(name="smp", bufs=4))
        ps = ctx.enter_context(tc.tile_pool(name="ps", bufs=2, space="PSUM"))

        ones_bf = const.tile([128, 64], BF16, tag="ones_bf")
        nc.vector.memset(ones_bf[:], 1.0)
        from concourse.masks import make_identity
        ident = const.tile([128, 128], F32, tag="ident")
        make_identity(nc, ident[:])
        ident_bf = const.tile([128, 128], BF16, tag="ident_bf")
        nc.vector.tensor_copy(ident_bf[:], ident[:])

        # ---- DMA issue order tracks consumption order ----
        pl = const.tile([1, D], F32, tag="pl")
        nc.sync.dma_start(pl[:], pl_d)
        bt = const.tile([1, H], F32, tag="bt")
        nc.sync.dma_start(bt[:], beta_d)
        bq_sb = const.tile([128, DC], F32, tag="bq_sb")
        nc.sync.dma_start(bq_sb[:], bq_d)
        bk_sb = const.tile([128, DC], F32, tag="bk_sb")
        nc.sync.dma_start(bk_sb[:], bk_d)
        bo_sb = const.tile([128, DC], F32, tag="bo_sb")
        nc.sync.dma_start(bo_sb[:], bo_d)

        # kg_value first: its PE transposes gate the k/v projections
        kv_tiles = []
        for c in range(KC):
            kv = kgkp.tile([128, D], BF16, tag="kgk", name="kv")
            nc.sync.dma_start(kv[:, 0:384], kgv_d[c * 128:(c + 1) * 128, 0:384])
            nc.sync.dma_start(kv[:, 384:768], kgv_d[c * 128:(c + 1) * 128, 384:768])
            kv_tiles.append(kv)

        wk_sb = []
        wv_sb = []
        wq_sb = []
        for c in range(DC):
            wk = wpool.tile([128, D], BF16, tag="w")
            nc.sync.dma_start(wk[:], wkt_d[c * 128:(c + 1) * 128, :])
            wk_sb.append(wk)
        # kg_key next: phase 0 (cosine bias, needed by the first exp)
        kgk_tiles = []
        for c in range(KC):
            kk = kgkp.tile([128, D], BF16, tag="kgk2", name="kgk")
            nc.sync.dma_start(kk[:, 0:384], kgk_d[c * 128:(c + 1) * 128, 0:384])
            nc.sync.dma_start(kk[:, 384:768], kgk_d[c * 128:(c + 1) * 128, 384:768])
            kgk_tiles.append(kk)


        # hs.T window 0 via the DMA xbar + wq: q-projection window 0
        hstw = [hstw_p.tile([128, T], BF16, tag="hstw", name="hstw")
                for _ in range(DC)]
        for c in range(DC):
            nc.sync.dma_start_transpose(
                hstw[c][:, 0:TW], hs_d[0:TW, c * 128:(c + 1) * 128])
        for c in range(DC):
            wq = wpool.tile([128, D], BF16, tag="w")
            nc.sync.dma_start(wq[:], wqt_d[c * 128:(c + 1) * 128, :])
            wq_sb.append(wq)
        for c in range(DC):
            wv = wpool.tile([128, D], BF16, tag="w")
            nc.sync.dma_start(wv[:], wvt_d[c * 128:(c + 1) * 128, :])
            wv_sb.append(wv)
        for tc4 in range(1, NTW):
            tws = slice(tc4 * TW, (tc4 + 1) * TW)
            for c in range(DC):
                nc.sync.dma_start_transpose(
                    hstw[c][:, tws], hs_d[tws, c * 128:(c + 1) * 128])
        wo_sb = []
        for c in range(DC):
            wo = wpool.tile([128, D], BF16, tag="w")
            nc.sync.dma_start(wo[:], wot_d[c * 128:(c + 1) * 128, :])
            wo_sb.append(wo)

        beta_bc = const.tile([128, H], F32, tag="beta_bc")
        nc.gpsimd.partition_broadcast(beta_bc[:], bt[:], channels=128)
        pl_bc = const.tile([128, D], F32, tag="pl_bc")
        nc.gpsimd.partition_broadcast(pl_bc[:], pl[:], channels=128)

        # kg_value.T via PE transposes (PE is otherwise idle this early)
        kgt = [kgt_p.tile([128, K], BF16, tag="kgt", name="kgt") for _ in range(DC)]
        for dchunk in range(DC):
            pt = ps.tile([128, K], BF16, tag="od", bufs=4, name="ptr")
            for c in range(KC):
                nc.tensor.transpose(
                    pt[:, c * 128:(c + 1) * 128],
                    kv_tiles[c][:, dchunk * 128:(dchunk + 1) * 128],
                    ident_bf[:])
            nc.vector.tensor_copy(kgt[dchunk][:], pt[:])

        # ---------------- phase 0: cosine-sim bias -> exp factor ----------------
        bias_all = const.tile([128, KC * H], F32, tag="bias_all")

        pl_sq = scr.tile([128, D], F32, tag="scr", name="pl_sq")
        pnorm = sm_p.tile([128, 1], F32, tag="pnorm")
        nc.scalar.activation(pl_sq[:], pl_bc[:], Act.Square, accum_out=pnorm[:])
        nc.scalar.activation(pnorm[:], pnorm[:], Act.Sqrt)
        nc.vector.tensor_scalar_max(pnorm[:], pnorm[:], EPS)
        rp_vec = const.tile([128, 1], F32, tag="rp_vec")
        nc.vector.reciprocal(rp_vec[:], pnorm[:])

        for c in range(KC):
            kk = kgk_tiles[c]
            sq = scr.tile([128, D], F32, tag="scr")
            nrm = sm_p.tile([128, 1], F32, tag="nrm")
            nc.scalar.activation(sq[:], kk[:], Act.Square, accum_out=nrm[:])
            nc.scalar.activation(nrm[:], nrm[:], Act.Sqrt)
            nc.vector.tensor_scalar_max(nrm[:], nrm[:], EPS)
            rn = sm_p.tile([128, 1], F32, tag="rn")
            nc.vector.reciprocal(rn[:], nrm[:])
            sq2 = scr.tile([128, D], F32, tag="scr")
            dot = sm_p.tile([128, 1], F32, tag="dot")
            nc.vector.scalar_tensor_tensor(
                out=sq2[:], in0=kk[:], scalar=1.0, in1=pl_bc[:],
                op0=Alu.mult, op1=Alu.mult, accum_out=dot[:])
            nc.vector.tensor_mul(dot[:], dot[:], rn[:])
            nc.vector.tensor_mul(dot[:], dot[:], rp_vec[:])
            nc.vector.tensor_scalar_mul(
                bias_all[:, c * H:(c + 1) * H], beta_bc[:], dot[:])

        # ---------------- phase 1a: k.T and v ----------------
        kt = [kt_p.tile([128, K], BF16, tag="kt", name="kt") for _ in range(DC)]
        for m in range(DC):
            pk = ps.tile([128, K], F32, tag="od", bufs=4)
            for c in range(DC):
                nc.tensor.matmul(
                    pk[:], wk_sb[c][:, m * 128:(m + 1) * 128], kgt[c][:],
                    start=(c == 0), stop=(c == DC - 1))
            nc.vector.tensor_scalar_add(kt[m][:], pk[:], bk_sb[:, m:m + 1])

        # ------- per t-window: q.T, then attention + final projection.
        # Fusing the q-projection into the window loop keeps the PE FIFO
        # from stalling on hs.T windows that are still in DMA flight. -------
        qt = [big.tile([128, T], BF16, tag="big", name="qt") for _ in range(DC)]
        ot = [big.tile([128, T], BF16, tag="big", name="ot") for _ in range(NPAIR)]

        def oproj_chain(wtb, m):
            ows = slice(wtb * TW, (wtb + 1) * TW)
            pf = ps.tile([128, TW], F32, tag="od", bufs=4, name="pf")
            for c in range(DC):
                nc.tensor.matmul(
                    pf[:], wo_sb[c][:, m * 128:(m + 1) * 128],
                    ot[c][:, ows],
                    start=(c == 0), stop=(c == DC - 1))
            fin = fin_p.tile([128, TW], F32, tag="fin")
            nc.vector.tensor_scalar_add(fin[:], pf[:], bo_sb[:, m:m + 1])
            nc.sync.dma_start(out_d[m * 128:(m + 1) * 128, ows], fin[:])

        def qproj_win(wtb):
            qws = slice(wtb * TW, (wtb + 1) * TW)
            for m in range(DC):
                pq = ps.tile([128, TW], F32, tag="s", bufs=4, name="pq")
                for c in range(DC):
                    nc.tensor.matmul(
                        pq[:], wq_sb[c][:, m * 128:(m + 1) * 128],
                        hstw[c][:, qws],
                        start=(c == 0), stop=(c == DC - 1))
                nc.vector.tensor_scalar_add(
                    qt[m][:, qws], pq[:], bq_sb[:, m:m + 1])

        qproj_win(0)
        v_sb = [v_p.tile([128, D], BF16, tag="v", name="vsb")
                for _ in range(KC)]
        for kc in range(KC):
            for n in range(2):
                pv = ps.tile([128, 384], F32, tag="od", bufs=4)
                for c in range(DC):
                    nc.tensor.matmul(
                        pv[:], kgt[c][:, kc * 128:(kc + 1) * 128],
                        wv_sb[c][:, n * 384:(n + 1) * 384],
                        start=(c == 0), stop=(c == DC - 1))
                nc.vector.tensor_copy(
                    v_sb[kc][:, n * 384:(n + 1) * 384], pv[:])


        for tb in range(NTW):
            tws = slice(tb * TW, (tb + 1) * TW)
            if tb >= 1:
                qproj_win(tb)

            for j in range(NPAIR):
                he = 2 * j
                ho = 2 * j + 1
                # scores (even/odd row-tiled concurrent pairs) + exp with
                # the per-partition cosine bias
                e_j = []
                for kc in range(KC):
                    h0 = kc * H + he
                    pse = ps.tile([128, TW], F32, tag="s", bufs=4)
                    nc.tensor.matmul(
                        pse[:],
                        kt[j][0:64, kc * 128:(kc + 1) * 128],
                        qt[j][0:64, tws], start=True, stop=True)
                    pso = ps.tile([128, TW], F32, tag="s", bufs=4)
                    nc.tensor.matmul(
                        pso[:],
                        kt[j][64:128, kc * 128:(kc + 1) * 128],
                        qt[j][64:128, tws], start=True, stop=True)
                    ee = e_p.tile([128, TW], BF16, tag="e")
                    nc.scalar.activation(
                        ee[:], pse[:], Act.Exp, bias=bias_all[:, h0:h0 + 1])
                    eo = e_p.tile([128, TW], BF16, tag="e")
                    nc.scalar.activation(
                        eo[:], pso[:], Act.Exp,
                        bias=bias_all[:, h0 + 1:h0 + 2])
                    e_j.append((ee, eo))

                # previous window's final-projection chain: ready PE work
                # that fills the exp-drain window before the AV chains
                if tb >= 1:
                    oproj_chain(tb - 1, j)

                # AV + denominator: po = [evenAV ; oddAV],
                # pd = [oddDen ; evenDen]; each v-matmul pairs with a
                # ones-matmul on the opposite col-group -> concurrent
                po = ps.tile([128, TW], F32, tag="od", bufs=4, name="po")
                pd = ps.tile([128, TW], F32, tag="od", bufs=4, name="pd")
                for kc in range(KC):
                    st = (kc == 0)
                    sp = (kc == KC - 1)
                    nc.tensor.matmul(
                        po[0:64, :],
                        v_sb[kc][:, he * HD:(he + 1) * HD],
                        e_j[kc][0][:], start=st, stop=sp)
                    nc.tensor.matmul(
                        pd[64:128, :], ones_bf[:, 0:64],
                        e_j[kc][0][:], start=st, stop=sp,
                        tile_position=(0, 64))
                for kc in range(KC):
                    st = (kc == 0)
                    sp = (kc == KC - 1)
                    nc.tensor.matmul(
                        po[64:128, :],
                        v_sb[kc][:, ho * HD:(ho + 1) * HD],
                        e_j[kc][1][:], start=st, stop=sp)
                    nc.tensor.matmul(
                        pd[0:64, :], ones_bf[:, 0:64],
                        e_j[kc][1][:], start=st, stop=sp,
                        tile_position=(0, 0))

                rall = r_p.tile([128, TW], F32, tag="rall", name="rall")
                nc.vector.reciprocal_approx_fast(rall[:], pd[:])
                nc.vector.tensor_mul(
                    ot[j][0:64, tws], po[0:64, :], rall[64:128, :])
                nc.vector.tensor_mul(
                    ot[j][64:128, tws], po[64:128, :], rall[0:64, :])

        for m in range(DC):
            oproj_chain(NTW - 1, m)

    nc.compile()
    return nc


def _get_program():
    if "nc" not in _CACHE:
        _CACHE["nc"] = _build_program()
    return _CACHE["nc"]


def _host_prep(inputs):
    import ml_dtypes
    bf16 = ml_dtypes.bfloat16

    f32 = lambda x: np.ascontiguousarray(np.asarray(x, dtype=np.float32))
    Wq, Wk, Wv, Wo = (f32(inputs[k]) for k in ("Wq", "Wk", "Wv", "Wo"))
    bq, bk, bv, bo = (f32(inputs[k]) for k in ("bq", "bk", "bv", "bo"))
    beta = f32(inputs["beta"])

    shared = {
        "wqt": np.ascontiguousarray((Wq.T * SCALE).astype(bf16)),
        "wkt": np.ascontiguousarray(Wk.T.astype(bf16)),
        "wvt": np.ascontiguousarray(Wv.T.astype(bf16)),
        "wot": np.ascontiguousarray(Wo.T.astype(bf16)),
        "bq": np.ascontiguousarray((bq * SCALE).reshape(DC, 128).T),
        "bk": np.ascontiguousarray(bk.reshape(DC, 128).T),
        # bv folded through Wo (sum_k softmax == 1), bo absorbed:
        "bo": np.ascontiguousarray((bo + bv @ Wo.T).reshape(DC, 128).T),
        "beta": np.ascontiguousarray(beta.reshape(1, H)),
    }

    hs = np.asarray(inputs["hidden_states"], dtype=np.float32)
    kgk = np.asarray(inputs["kg_key"], dtype=np.float32)
    kgv = np.asarray(inputs["kg_value"], dtype=np.float32)
    pooled = f32(inputs["pooled_hidden_states"])

    in_maps = []
    for b in range(BS):
        m = dict(shared)
        m["hs"] = np.ascontiguousarray(hs[b].astype(bf16))
        m["kgk"] = np.ascontiguousarray(kgk[b].astype(bf16))
        m["kgv"] = np.ascontiguousarray(kgv[b].astype(bf16))
        m["pooled"] = np.ascontiguousarray(pooled[b].reshape(1, D))
        in_maps.append(m)
    return in_maps




def _install_ntff_hook():
    """Register the axon NTFF profile hook so trace=True yields exec_time_ns.

    Only used from our own test harness (TRACE=True); the default kernel()
    path never calls this.
    """
    try:
        from antenv.axon_hooks import get_axon_ntff_profile_hook  # noqa: F401
        return
    except ImportError:
        pass
    import contextlib
    import ctypes
    import types

    so_path = "/opt/axon/libaxon_pjrt.so"
    try:
        lib = ctypes.CDLL(so_path)
    except OSError:
        return
    if not hasattr(lib, "axon_start_nrt_profile"):
        return
    lib.axon_start_nrt_profile.argtypes = [
        ctypes.POINTER(ctypes.c_int64), ctypes.c_size_t]
    lib.axon_start_nrt_profile.restype = ctypes.c_int64
    lib.axon_stop_nrt_profile.argtypes = [ctypes.c_char_p]
    lib.axon_stop_nrt_profile.restype = ctypes.c_int64

    @contextlib.contextmanager
    def _hook(output_dir, device_ids):
        import jax
        jax.devices()
        if device_ids:
            ids = (ctypes.c_int64 * len(device_ids))(*device_ids)
            rc = lib.axon_start_nrt_profile(ids, len(device_ids))
        else:
            rc = lib.axon_start_nrt_profile(None, 0)
        if rc != 0:
            raise RuntimeError(f"axon_start_nrt_profile rc={rc}")
        try:
            yield
        finally:
            n = lib.axon_stop_nrt_profile(str(output_dir).encode())
            print(f"profile: {n} file(s) written to {output_dir}",
                  file=sys.stderr)

    mod = types.ModuleType("antenv.axon_hooks")
    mod.get_axon_ntff_profile_hook = lambda: _hook
    mod.set_axon_ntff_profile_hook = lambda h: None
    sys.modules["antenv.axon_hooks"] = mod


def kernel(**inputs):
    global LAST_EXEC_NS
    _ensure_path()
    from concourse import bass_utils

    if TRACE:
        _install_ntff_hook()
    nc = _get_program()
    in_maps = _host_prep(inputs)
    res = bass_utils.run_bass_kernel_spmd(
        nc, in_maps, core_ids=list(range(BS)), trace=TRACE)
    LAST_EXEC_NS = res.exec_time_ns
    # device output is out.T [D, T]; un-transpose per example
    out = np.stack([res.results[b]["out"].T for b in range(BS)], axis=0)
    return np.ascontiguousarray(out).astype(np.float32)


# revision 29
# speedup vs baseline: 1.0179x; 1.0179x over previous
"""Trainium2 Bass kernel for nn_KnowledgeAttention.

Math (per batch example b):
    sim[k]  = cos_sim(pooled[b], kg_key[b,k])                      # [K]
    q       = (hs @ Wq.T + bq) * HD**-0.5     -> heads [T,H,HD]
    k       = kg_value @ Wk.T + bk            -> heads [K,H,HD]
    v       = kg_value @ Wv.T + bv            -> heads [K,H,HD]
    S[h,t,k]= q_h[t]·k_h[k] + beta[h]*sim[k]
    P       = softmax_k(S);  O[t,h] = sum_k P v
    out     = O @ Wo.T + bo

Sharding: pure data-parallel over batch — 8 examples on 8 cores, weights
replicated, no collectives.

Per-core design notes:
  * hs / kg_value / kg_key are shipped bf16 from the host; hs.T and
    kg_value.T land in SBUF directly via DMA-xbar transpose loads
    (dma_start_transpose) — no PE transposes, no psum->sbuf copies.
  * the per-head cosine bias is FACTORED OUT of the exp:
    exp(S + b) = exp(S) * exp(b); exp(b) is folded into v (applied
    during the v-projection psum->sbuf copy, on the scalar engine with
    a per-partition scale) and into the denominator matmul stationary
    (ebrep = exp(b) replicated 64 wide).  The exp is then bias-free, so
    one ACT op covers the even AND odd head of a pair ([e|o]-batched,
    1024 wide across two psum banks).
  * scores are computed S.T[k,t] as even/odd row-tiled concurrent
    matmul pairs (stationaries at partition 0:64 / 64:128).
  * AV chains: po = [evenAV ; oddAV]; the denominator matmuls (ebrep
    stationary) go to the opposite array col-group via tile_position so
    they stream the same e tile concurrently with the v matmuls.
  * final projection computed transposed (out.T[dout,t]) so the moving
    operand is ot directly; DRAM output is [D,T], un-transposed on host.
  * matmuls in bf16 with fp32 PSUM accumulation.
"""

import sys

import numpy as np

# ---------------------------------------------------------------- constants
BS = 8
T = 2048
D = 768
H = 12
HD = 64
K = 512
SCALE = HD ** -0.5
EPS = 1e-8
DC = D // 128   # 6 contraction/partition chunks of 128 over D
KC = K // 128   # 4 chunks over K
TW = 512        # t window (psum free-dim limit)
NTW = T // TW   # 4
NPAIR = H // 2  # 6 head pairs

TRACE = False
LAST_EXEC_NS = None

_CACHE = {}


def _ensure_path():
    try:
        import concourse  # noqa: F401
    except ImportError:
        for p in ("/opt/trn_rl_repo", "/root/.axon_site/_ro/trn_rl_repo"):
            if p not in sys.path:
                sys.path.insert(0, p)


def _build_program():
    _ensure_path()
    import concourse.bass as bass
    import concourse.mybir as mybir
    import concourse.tile as tile
    from concourse import bacc
    from contextlib import ExitStack

    F32 = mybir.dt.float32
    BF16 = mybir.dt.bfloat16
    Alu = mybir.AluOpType
    Act = mybir.ActivationFunctionType

    nc = bacc.Bacc("TRN2", target_bir_lowering=False, debug=False, num_devices=BS)

    hs_d = nc.dram_tensor("hs", [T, D], BF16, kind="ExternalInput").ap()
    kgk_d = nc.dram_tensor("kgk", [K, D], BF16, kind="ExternalInput").ap()
    kgv_d = nc.dram_tensor("kgv", [K, D], BF16, kind="ExternalInput").ap()
    pl_d = nc.dram_tensor("pooled", [1, D], F32, kind="ExternalInput").ap()
    wqt_d = nc.dram_tensor("wqt", [D, D], BF16, kind="ExternalInput").ap()
    wkt_d = nc.dram_tensor("wkt", [D, D], BF16, kind="ExternalInput").ap()
    wvt_d = nc.dram_tensor("wvt", [D, D], BF16, kind="ExternalInput").ap()
    wot_d = nc.dram_tensor("wot", [D, D], BF16, kind="ExternalInput").ap()
    bq_d = nc.dram_tensor("bq", [128, DC], F32, kind="ExternalInput").ap()
    bk_d = nc.dram_tensor("bk", [128, DC], F32, kind="ExternalInput").ap()
    bo_d = nc.dram_tensor("bo", [128, DC], F32, kind="ExternalInput").ap()
    beta_d = nc.dram_tensor("beta", [1, H], F32, kind="ExternalInput").ap()
    # output stored transposed [D, T]; host un-transposes
    out_d = nc.dram_tensor("out", [D, T], F32, kind="ExternalOutput").ap()

    with tile.TileContext(nc) as tc, ExitStack() as ctx:
        const = ctx.enter_context(tc.tile_pool(name="const", bufs=1))
        kgkp = ctx.enter_context(tc.tile_pool(name="kgkp", bufs=4))
        scr = ctx.enter_context(tc.tile_pool(name="scr", bufs=3))
        wpool = ctx.enter_context(tc.tile_pool(name="w", bufs=18))
        big = ctx.enter_context(tc.tile_pool(name="big", bufs=12))
        hstw_p = ctx.enter_context(tc.tile_pool(name="hstw", bufs=6))
        kt_p = ctx.enter_context(tc.tile_pool(name="ktp", bufs=6))
        v_p = ctx.enter_context(tc.tile_pool(name="vp", bufs=4))
        kgt_p = ctx.enter_context(tc.tile_pool(name="kgtp", bufs=6))
        e_p = ctx.enter_context(tc.tile_pool(name="ep", bufs=12))
        r_p = ctx.enter_context(tc.tile_pool(name="rp", bufs=4))
        fin_p = ctx.enter_context(tc.tile_pool(name="finp", bufs=6))
        sm_p = ctx.enter_context(tc.tile_pool(name="smp", bufs=4))
        ps = ctx.enter_context(tc.tile_pool(name="ps", bufs=2, space="PSUM"))

        ones_bf = const.tile([128, 64], BF16, tag="ones_bf")
        nc.vector.memset(ones_bf[:], 1.0)
        from concourse.masks import make_identity
        ident = const.tile([128, 128], F32, tag="ident")
        make_identity(nc, ident[:])
        ident_bf = const.tile([128, 128], BF16, tag="ident_bf")
        nc.vector.tensor_copy(ident_bf[:], ident[:])

        # ---- DMA issue order tracks consumption order ----
        pl = const.tile([1, D], F32, tag="pl")
        nc.sync.dma_start(pl[:], pl_d)
        bt = const.tile([1, H], F32, tag="bt")
        nc.sync.dma_start(bt[:], beta_d)
        bq_sb = const.tile([128, DC], F32, tag="bq_sb")
        nc.sync.dma_start(bq_sb[:], bq_d)
        bk_sb = const.tile([128, DC], F32, tag="bk_sb")
        nc.sync.dma_start(bk_sb[:], bk_d)
        bo_sb = const.tile([128, DC], F32, tag="bo_sb")
        nc.sync.dma_start(bo_sb[:], bo_d)

        # kg_value first: its PE transposes gate the k/v projections
        kv_tiles = []
        for c in range(KC):
            kv = kgkp.tile([128, D], BF16, tag="kgk", name="kv")
            nc.sync.dma_start(kv[:, 0:384], kgv_d[c * 128:(c + 1) * 128, 0:384])
            nc.sync.dma_start(kv[:, 384:768], kgv_d[c * 128:(c + 1) * 128, 384:768])
            kv_tiles.append(kv)

        wk_sb = []
        wv_sb = []
        wq_sb = []
        for c in range(DC):
            wk = wpool.tile([128, D], BF16, tag="w")
            nc.sync.dma_start(wk[:], wkt_d[c * 128:(c + 1) * 128, :])
            wk_sb.append(wk)
        # hs.T window 0 via the DMA xbar + wq: q-projection window 0
        hstw = [hstw_p.tile([128, T], BF16, tag="hstw", name="hstw")
                for _ in range(DC)]
        for c in range(DC):
            nc.sync.dma_start_transpose(
                hstw[c][:, 0:TW], hs_d[0:TW, c * 128:(c + 1) * 128])
        for c in range(DC):
            wq = wpool.tile([128, D], BF16, tag="w")
            nc.sync.dma_start(wq[:], wqt_d[c * 128:(c + 1) * 128, :])
            wq_sb.append(wq)
        # kg_key next: phase 0 (cosine bias, needed by the first exp)
        kgk_tiles = []
        for c in range(KC):
            kk = kgkp.tile([128, D], BF16, tag="kgk2", name="kgk")
            nc.sync.dma_start(kk[:, 0:384], kgk_d[c * 128:(c + 1) * 128, 0:384])
            nc.sync.dma_start(kk[:, 384:768], kgk_d[c * 128:(c + 1) * 128, 384:768])
            kgk_tiles.append(kk)


        for c in range(DC):
            wv = wpool.tile([128, D], BF16, tag="w")
            nc.sync.dma_start(wv[:], wvt_d[c * 128:(c + 1) * 128, :])
            wv_sb.append(wv)
        for tc4 in range(1, NTW):
            tws = slice(tc4 * TW, (tc4 + 1) * TW)
            for c in range(DC):
                nc.sync.dma_start_transpose(
                    hstw[c][:, tws], hs_d[tws, c * 128:(c + 1) * 128])
        wo_sb = []
        for c in range(DC):
            wo = wpool.tile([128, D], BF16, tag="w")
            nc.sync.dma_start(wo[:], wot_d[c * 128:(c + 1) * 128, :])
            wo_sb.append(wo)

        beta_bc = const.tile([128, H], F32, tag="beta_bc")
        nc.gpsimd.partition_broadcast(beta_bc[:], bt[:], channels=128)
        pl_bc = const.tile([128, D], F32, tag="pl_bc")
        nc.gpsimd.partition_broadcast(pl_bc[:], pl[:], channels=128)

        # kg_value.T via PE transposes (PE is otherwise idle this early)
        kgt = [kgt_p.tile([128, K], BF16, tag="kgt", name="kgt") for _ in range(DC)]
        for dchunk in range(DC):
            pt = ps.tile([128, K], BF16, tag="od", bufs=4, name="ptr")
            for c in range(KC):
                nc.tensor.transpose(
                    pt[:, c * 128:(c + 1) * 128],
                    kv_tiles[c][:, dchunk * 128:(dchunk + 1) * 128],
                    ident_bf[:])
            nc.vector.tensor_copy(kgt[dchunk][:], pt[:])

        # ---------------- phase 0: cosine-sim bias -> exp factor ----------------
        bias_all = const.tile([128, KC * H], F32, tag="bias_all")

        pl_sq = scr.tile([128, D], F32, tag="scr", name="pl_sq")
        pnorm = sm_p.tile([128, 1], F32, tag="pnorm")
        nc.scalar.activation(pl_sq[:], pl_bc[:], Act.Square, accum_out=pnorm[:])
        nc.scalar.activation(pnorm[:], pnorm[:], Act.Sqrt)
        nc.vector.tensor_scalar_max(pnorm[:], pnorm[:], EPS)
        rp_vec = const.tile([128, 1], F32, tag="rp_vec")
        nc.vector.reciprocal(rp_vec[:], pnorm[:])

        for c in range(KC):
            kk = kgk_tiles[c]
            sq = scr.tile([128, D], F32, tag="scr")
            nrm = sm_p.tile([128, 1], F32, tag="nrm")
            nc.scalar.activation(sq[:], kk[:], Act.Square, accum_out=nrm[:])
            nc.scalar.activation(nrm[:], nrm[:], Act.Sqrt)
            nc.vector.tensor_scalar_max(nrm[:], nrm[:], EPS)
            rn = sm_p.tile([128, 1], F32, tag="rn")
            nc.vector.reciprocal(rn[:], nrm[:])
            sq2 = scr.tile([128, D], F32, tag="scr")
            dot = sm_p.tile([128, 1], F32, tag="dot")
            nc.vector.scalar_tensor_tensor(
                out=sq2[:], in0=kk[:], scalar=1.0, in1=pl_bc[:],
                op0=Alu.mult, op1=Alu.mult, accum_out=dot[:])
            nc.vector.tensor_mul(dot[:], dot[:], rn[:])
            nc.vector.tensor_mul(dot[:], dot[:], rp_vec[:])
            nc.vector.tensor_scalar_mul(
                bias_all[:, c * H:(c + 1) * H], beta_bc[:], dot[:])

        # ---------------- phase 1a: k.T and v ----------------
        kt = [kt_p.tile([128, K], BF16, tag="kt", name="kt") for _ in range(DC)]
        for m in range(DC):
            pk = ps.tile([128, K], F32, tag="od", bufs=4)
            for c in range(DC):
                nc.tensor.matmul(
                    pk[:], wk_sb[c][:, m * 128:(m + 1) * 128], kgt[c][:],
                    start=(c == 0), stop=(c == DC - 1))
            nc.vector.tensor_scalar_add(kt[m][:], pk[:], bk_sb[:, m:m + 1])

        # ------- per t-window: q.T, then attention + final projection.
        # Fusing the q-projection into the window loop keeps the PE FIFO
        # from stalling on hs.T windows that are still in DMA flight. -------
        qt = [big.tile([128, T], BF16, tag="big", name="qt") for _ in range(DC)]
        ot = [big.tile([128, T], BF16, tag="big", name="ot") for _ in range(NPAIR)]

        def oproj_chain(wtb, m):
            ows = slice(wtb * TW, (wtb + 1) * TW)
            pf = ps.tile([128, TW], F32, tag="od", bufs=4, name="pf")
            for c in range(DC):
                nc.tensor.matmul(
                    pf[:], wo_sb[c][:, m * 128:(m + 1) * 128],
                    ot[c][:, ows],
                    start=(c == 0), stop=(c == DC - 1))
            fin = fin_p.tile([128, TW], F32, tag="fin")
            nc.vector.tensor_scalar_add(fin[:], pf[:], bo_sb[:, m:m + 1])
            nc.sync.dma_start(out_d[m * 128:(m + 1) * 128, ows], fin[:])

        def qproj_win(wtb):
            qws = slice(wtb * TW, (wtb + 1) * TW)
            for m in range(DC):
                pq = ps.tile([128, TW], F32, tag="s", bufs=4, name="pq")
                for c in range(DC):
                    nc.tensor.matmul(
                        pq[:], wq_sb[c][:, m * 128:(m + 1) * 128],
                        hstw[c][:, qws],
                        start=(c == 0), stop=(c == DC - 1))
                nc.vector.tensor_scalar_add(
                    qt[m][:, qws], pq[:], bq_sb[:, m:m + 1])

        qproj_win(0)
        v_sb = [v_p.tile([128, D], BF16, tag="v", name="vsb")
                for _ in range(KC)]
        for kc in range(KC):
            for n in range(2):
                pv = ps.tile([128, 384], F32, tag="od", bufs=4)
                for c in range(DC):
                    nc.tensor.matmul(
                        pv[:], kgt[c][:, kc * 128:(kc + 1) * 128],
                        wv_sb[c][:, n * 384:(n + 1) * 384],
                        start=(c == 0), stop=(c == DC - 1))
                nc.vector.tensor_copy(
                    v_sb[kc][:, n * 384:(n + 1) * 384], pv[:])


        for tb in range(NTW):
            tws = slice(tb * TW, (tb + 1) * TW)
            if tb >= 1:
                qproj_win(tb)

            for j in range(NPAIR):
                he = 2 * j
                ho = 2 * j + 1
                # previous window's final-projection chain, its matmuls
                # interleaved between the score pairs: ready PE work that
                # fills the exp-drain pacing holes
                if tb >= 1:
                    ows = slice((tb - 1) * TW, tb * TW)
                    pf = ps.tile([128, TW], F32, tag="od", bufs=4, name="pf")
                # scores (even/odd row-tiled concurrent pairs) + exp with
                # the per-partition cosine bias
                e_j = []
                for kc in range(KC):
                    h0 = kc * H + he
                    pse = ps.tile([128, TW], F32, tag="s", bufs=4)
                    nc.tensor.matmul(
                        pse[:],
                        kt[j][0:64, kc * 128:(kc + 1) * 128],
                        qt[j][0:64, tws], start=True, stop=True)
                    pso = ps.tile([128, TW], F32, tag="s", bufs=4)
                    nc.tensor.matmul(
                        pso[:],
                        kt[j][64:128, kc * 128:(kc + 1) * 128],
                        qt[j][64:128, tws], start=True, stop=True)
                    if tb >= 1:
                        for c in (2 * kc, 2 * kc + 1):
                            if c < DC:
                                nc.tensor.matmul(
                                    pf[:],
                                    wo_sb[c][:, j * 128:(j + 1) * 128],
                                    ot[c][:, ows],
                                    start=(c == 0), stop=(c == DC - 1))
                    ee = e_p.tile([128, TW], BF16, tag="e")
                    nc.scalar.activation(
                        ee[:], pse[:], Act.Exp, bias=bias_all[:, h0:h0 + 1])
                    eo = e_p.tile([128, TW], BF16, tag="e")
                    nc.scalar.activation(
                        eo[:], pso[:], Act.Exp,
                        bias=bias_all[:, h0 + 1:h0 + 2])
                    e_j.append((ee, eo))

                if tb >= 1:
                    fin = fin_p.tile([128, TW], F32, tag="fin")
                    nc.vector.tensor_scalar_add(
                        fin[:], pf[:], bo_sb[:, j:j + 1])
                    nc.sync.dma_start(
                        out_d[j * 128:(j + 1) * 128, ows], fin[:])

                # AV + denominator: po = [evenAV ; oddAV],
                # pd = [oddDen ; evenDen]; each v-matmul pairs with a
                # ones-matmul on the opposite col-group -> concurrent
                po = ps.tile([128, TW], F32, tag="od", bufs=4, name="po")
                pd = ps.tile([128, TW], F32, tag="od", bufs=4, name="pd")
                for kc in range(KC):
                    st = (kc == 0)
                    sp = (kc == KC - 1)
                    nc.tensor.matmul(
                        po[0:64, :],
                        v_sb[kc][:, he * HD:(he + 1) * HD],
                        e_j[kc][0][:], start=st, stop=sp)
                    nc.tensor.matmul(
                        pd[64:128, :], ones_bf[:, 0:64],
                        e_j[kc][0][:], start=st, stop=sp,
                        tile_position=(0, 64))
                for kc in range(KC):
                    st = (kc == 0)
                    sp = (kc == KC - 1)
                    nc.tensor.matmul(
                        po[64:128, :],
                        v_sb[kc][:, ho * HD:(ho + 1) * HD],
                        e_j[kc][1][:], start=st, stop=sp)
                    nc.tensor.matmul(
                        pd[0:64, :], ones_bf[:, 0:64],
                        e_j[kc][1][:], start=st, stop=sp,
                        tile_position=(0, 0))

                rall = r_p.tile([128, TW], F32, tag="rall", name="rall")
                nc.vector.reciprocal_approx_fast(rall[:], pd[:])
                nc.vector.tensor_mul(
                    ot[j][0:64, tws], po[0:64, :], rall[64:128, :])
                nc.vector.tensor_mul(
                    ot[j][64:128, tws], po[64:128, :], rall[0:64, :])

        for m in range(DC):
            oproj_chain(NTW - 1, m)

    nc.compile()
    return nc


def _get_program():
    if "nc" not in _CACHE:
        _CACHE["nc"] = _build_program()
    return _CACHE["nc"]


def _host_prep(inputs):
    import ml_dtypes
    bf16 = ml_dtypes.bfloat16

    f32 = lambda x: np.ascontiguousarray(np.asarray(x, dtype=np.float32))
    Wq, Wk, Wv, Wo = (f32(inputs[k]) for k in ("Wq", "Wk", "Wv", "Wo"))
    bq, bk, bv, bo = (f32(inputs[k]) for k in ("bq", "bk", "bv", "bo"))
    beta = f32(inputs["beta"])

    shared = {
        "wqt": np.ascontiguousarray((Wq.T * SCALE).astype(bf16)),
        "wkt": np.ascontiguousarray(Wk.T.astype(bf16)),
        "wvt": np.ascontiguousarray(Wv.T.astype(bf16)),
        "wot": np.ascontiguousarray(Wo.T.astype(bf16)),
        "bq": np.ascontiguousarray((bq * SCALE).reshape(DC, 128).T),
        "bk": np.ascontiguousarray(bk.reshape(DC, 128).T),
        # bv folded through Wo (sum_k softmax == 1), bo absorbed:
        "bo": np.ascontiguousarray((bo + bv @ Wo.T).reshape(DC, 128).T),
        "beta": np.ascontiguousarray(beta.reshape(1, H)),
    }

    hs = np.asarray(inputs["hidden_states"], dtype=np.float32)
    kgk = np.asarray(inputs["kg_key"], dtype=np.float32)
    kgv = np.asarray(inputs["kg_value"], dtype=np.float32)
    pooled = f32(inputs["pooled_hidden_states"])

    in_maps = []
    for b in range(BS):
        m = dict(shared)
        m["hs"] = np.ascontiguousarray(hs[b].astype(bf16))
        m["kgk"] = np.ascontiguousarray(kgk[b].astype(bf16))
        m["kgv"] = np.ascontiguousarray(kgv[b].astype(bf16))
        m["pooled"] = np.ascontiguousarray(pooled[b].reshape(1, D))
        in_maps.append(m)
    return in_maps




def _install_ntff_hook():
    """Register the axon NTFF profile hook so trace=True yields exec_time_ns.

    Only used from our own test harness (TRACE=True); the default kernel()
    path never calls this.
    """
    try:
        from antenv.axon_hooks import get_axon_ntff_profile_hook  # noqa: F401
        return
    except ImportError:
        pass
    import contextlib
    import ctypes
    import types

    so_path = "/opt/axon/libaxon_pjrt.so"
    try:
        lib = ctypes.CDLL(so_path)
    except OSError:
        return
    if not hasattr(lib, "axon_start_nrt_profile"):
        return
    lib.axon_start_nrt_profile.argtypes = [
        ctypes.POINTER(ctypes.c_int64), ctypes.c_size_t]
    lib.axon_start_nrt_profile.restype = ctypes.c_int64
    lib.axon_stop_nrt_profile.argtypes = [ctypes.c_char_p]
    lib.axon_stop_nrt_profile.restype = ctypes.c_int64

    @contextlib.contextmanager
    def _hook(output_dir, device_ids):
        import jax
        jax.devices()
        if device_ids:
            ids = (ctypes.c_int64 * len(device_ids))(*device_ids)
            rc = lib.axon_start_nrt_profile(ids, len(device_ids))
        else:
            rc = lib.axon_start_nrt_profile(None, 0)
        if rc != 0:
            raise RuntimeError(f"axon_start_nrt_profile rc={rc}")
        try:
            yield
        finally:
            n = lib.axon_stop_nrt_profile(str(output_dir).encode())
            print(f"profile: {n} file(s) written to {output_dir}",
                  file=sys.stderr)

    mod = types.ModuleType("antenv.axon_hooks")
    mod.get_axon_ntff_profile_hook = lambda: _hook
    mod.set_axon_ntff_profile_hook = lambda h: None
    sys.modules["antenv.axon_hooks"] = mod


def kernel(**inputs):
    global LAST_EXEC_NS
    _ensure_path()
    from concourse import bass_utils

    if TRACE:
        _install_ntff_hook()
    nc = _get_program()
    in_maps = _host_prep(inputs)
    res = bass_utils.run_bass_kernel_spmd(
        nc, in_maps, core_ids=list(range(BS)), trace=TRACE)
    LAST_EXEC_NS = res.exec_time_ns
    # device output is out.T [D, T]; un-transpose per example
    out = np.stack([res.results[b]["out"].T for b in range(BS)], axis=0)
    return np.ascontiguousarray(out).astype(np.float32)


# revision 30
# speedup vs baseline: 1.0182x; 1.0003x over previous
"""Trainium2 Bass kernel for nn_KnowledgeAttention.

Math (per batch example b):
    sim[k]  = cos_sim(pooled[b], kg_key[b,k])                      # [K]
    q       = (hs @ Wq.T + bq) * HD**-0.5     -> heads [T,H,HD]
    k       = kg_value @ Wk.T + bk            -> heads [K,H,HD]
    v       = kg_value @ Wv.T + bv            -> heads [K,H,HD]
    S[h,t,k]= q_h[t]·k_h[k] + beta[h]*sim[k]
    P       = softmax_k(S);  O[t,h] = sum_k P v
    out     = O @ Wo.T + bo

Sharding: pure data-parallel over batch — 8 examples on 8 cores, weights
replicated, no collectives.

Per-core design notes:
  * hs / kg_value / kg_key are shipped bf16 from the host; hs.T and
    kg_value.T land in SBUF directly via DMA-xbar transpose loads
    (dma_start_transpose) — no PE transposes, no psum->sbuf copies.
  * the per-head cosine bias is FACTORED OUT of the exp:
    exp(S + b) = exp(S) * exp(b); exp(b) is folded into v (applied
    during the v-projection psum->sbuf copy, on the scalar engine with
    a per-partition scale) and into the denominator matmul stationary
    (ebrep = exp(b) replicated 64 wide).  The exp is then bias-free, so
    one ACT op covers the even AND odd head of a pair ([e|o]-batched,
    1024 wide across two psum banks).
  * scores are computed S.T[k,t] as even/odd row-tiled concurrent
    matmul pairs (stationaries at partition 0:64 / 64:128).
  * AV chains: po = [evenAV ; oddAV]; the denominator matmuls (ebrep
    stationary) go to the opposite array col-group via tile_position so
    they stream the same e tile concurrently with the v matmuls.
  * final projection computed transposed (out.T[dout,t]) so the moving
    operand is ot directly; DRAM output is [D,T], un-transposed on host.
  * matmuls in bf16 with fp32 PSUM accumulation.
"""

import sys

import numpy as np

# ---------------------------------------------------------------- constants
BS = 8
T = 2048
D = 768
H = 12
HD = 64
K = 512
SCALE = HD ** -0.5
EPS = 1e-8
DC = D // 128   # 6 contraction/partition chunks of 128 over D
KC = K // 128   # 4 chunks over K
TW = 512        # t window (psum free-dim limit)
NTW = T // TW   # 4
NPAIR = H // 2  # 6 head pairs

TRACE = False
LAST_EXEC_NS = None

_CACHE = {}


def _ensure_path():
    try:
        import concourse  # noqa: F401
    except ImportError:
        for p in ("/opt/trn_rl_repo", "/root/.axon_site/_ro/trn_rl_repo"):
            if p not in sys.path:
                sys.path.insert(0, p)


def _build_program():
    _ensure_path()
    import concourse.bass as bass
    import concourse.mybir as mybir
    import concourse.tile as tile
    from concourse import bacc
    from contextlib import ExitStack

    F32 = mybir.dt.float32
    BF16 = mybir.dt.bfloat16
    Alu = mybir.AluOpType
    Act = mybir.ActivationFunctionType

    nc = bacc.Bacc("TRN2", target_bir_lowering=False, debug=False, num_devices=BS)

    hs_d = nc.dram_tensor("hs", [T, D], BF16, kind="ExternalInput").ap()
    kgk_d = nc.dram_tensor("kgk", [K, D], BF16, kind="ExternalInput").ap()
    kgv_d = nc.dram_tensor("kgv", [K, D], BF16, kind="ExternalInput").ap()
    pl_d = nc.dram_tensor("pooled", [1, D], F32, kind="ExternalInput").ap()
    wqt_d = nc.dram_tensor("wqt", [D, D], BF16, kind="ExternalInput").ap()
    wkt_d = nc.dram_tensor("wkt", [D, D], BF16, kind="ExternalInput").ap()
    wvt_d = nc.dram_tensor("wvt", [D, D], BF16, kind="ExternalInput").ap()
    wot_d = nc.dram_tensor("wot", [D, D], BF16, kind="ExternalInput").ap()
    bq_d = nc.dram_tensor("bq", [128, DC], F32, kind="ExternalInput").ap()
    bk_d = nc.dram_tensor("bk", [128, DC], F32, kind="ExternalInput").ap()
    bo_d = nc.dram_tensor("bo", [128, DC], F32, kind="ExternalInput").ap()
    beta_d = nc.dram_tensor("beta", [1, H], F32, kind="ExternalInput").ap()
    # output stored transposed [D, T]; host un-transposes
    out_d = nc.dram_tensor("out", [D, T], F32, kind="ExternalOutput").ap()

    with tile.TileContext(nc) as tc, ExitStack() as ctx:
        const = ctx.enter_context(tc.tile_pool(name="const", bufs=1))
        kgkp = ctx.enter_context(tc.tile_pool(name="kgkp", bufs=4))
        scr = ctx.enter_context(tc.tile_pool(name="scr", bufs=3))
        wpool = ctx.enter_context(tc.tile_pool(name="w", bufs=18))
        big = ctx.enter_context(tc.tile_pool(name="big", bufs=12))
        hstw_p = ctx.enter_context(tc.tile_pool(name="hstw", bufs=6))
        kt_p = ctx.enter_context(tc.tile_pool(name="ktp", bufs=6))
        v_p = ctx.enter_context(tc.tile_pool(name="vp", bufs=4))
        kgt_p = ctx.enter_context(tc.tile_pool(name="kgtp", bufs=6))
        e_p = ctx.enter_context(tc.tile_pool(name="ep", bufs=12))
        r_p = ctx.enter_context(tc.tile_pool(name="rp", bufs=4))
        fin_p = ctx.enter_context(tc.tile_pool(name="finp", bufs=6))
        sm_p = ctx.enter_context(tc.tile_pool(name="smp", bufs=4))
        ps = ctx.enter_context(tc.tile_pool(name="ps", bufs=2, space="PSUM"))

        ones_bf = const.tile([128, 64], BF16, tag="ones_bf")
        nc.vector.memset(ones_bf[:], 1.0)
        from concourse.masks import make_identity
        ident = const.tile([128, 128], F32, tag="ident")
        make_identity(nc, ident[:])
        ident_bf = const.tile([128, 128], BF16, tag="ident_bf")
        nc.vector.tensor_copy(ident_bf[:], ident[:])

        # ---- DMA issue order tracks consumption order ----
        pl = const.tile([1, D], F32, tag="pl")
        nc.sync.dma_start(pl[:], pl_d)
        bt = const.tile([1, H], F32, tag="bt")
        nc.sync.dma_start(bt[:], beta_d)
        bq_sb = const.tile([128, DC], F32, tag="bq_sb")
        nc.sync.dma_start(bq_sb[:], bq_d)
        bk_sb = const.tile([128, DC], F32, tag="bk_sb")
        nc.sync.dma_start(bk_sb[:], bk_d)
        bo_sb = const.tile([128, DC], F32, tag="bo_sb")
        nc.sync.dma_start(bo_sb[:], bo_d)

        # kg_value first: its PE transposes gate the k/v projections
        kv_tiles = []
        for c in range(KC):
            kv = kgkp.tile([128, D], BF16, tag="kgk", name="kv")
            nc.sync.dma_start(kv[:, 0:384], kgv_d[c * 128:(c + 1) * 128, 0:384])
            nc.sync.dma_start(kv[:, 384:768], kgv_d[c * 128:(c + 1) * 128, 384:768])
            kv_tiles.append(kv)

        wk_sb = []
        wv_sb = []
        wq_sb = []
        for c in range(DC):
            wk = wpool.tile([128, D], BF16, tag="w")
            nc.sync.dma_start(wk[:], wkt_d[c * 128:(c + 1) * 128, :])
            wk_sb.append(wk)
        # hs.T window 0 via the DMA xbar + wq: q-projection window 0
        hstw = [hstw_p.tile([128, T], BF16, tag="hstw", name="hstw")
                for _ in range(DC)]
        for c in range(DC):
            nc.sync.dma_start_transpose(
                hstw[c][:, 0:TW], hs_d[0:TW, c * 128:(c + 1) * 128])
        for c in range(DC):
            wq = wpool.tile([128, D], BF16, tag="w")
            nc.sync.dma_start(wq[:], wqt_d[c * 128:(c + 1) * 128, :])
            wq_sb.append(wq)
        # kg_key next: phase 0 (cosine bias, needed by the first exp)
        kgk_tiles = []
        for c in range(KC):
            kk = kgkp.tile([128, D], BF16, tag="kgk2", name="kgk")
            nc.sync.dma_start(kk[:, 0:384], kgk_d[c * 128:(c + 1) * 128, 0:384])
            nc.sync.dma_start(kk[:, 384:768], kgk_d[c * 128:(c + 1) * 128, 384:768])
            kgk_tiles.append(kk)


        for c in range(DC):
            wv = wpool.tile([128, D], BF16, tag="w")
            nc.sync.dma_start(wv[:], wvt_d[c * 128:(c + 1) * 128, :])
            wv_sb.append(wv)
        for tc4 in range(1, NTW):
            tws = slice(tc4 * TW, (tc4 + 1) * TW)
            for c in range(DC):
                nc.sync.dma_start_transpose(
                    hstw[c][:, tws], hs_d[tws, c * 128:(c + 1) * 128])
        wo_sb = []
        for c in range(DC):
            wo = wpool.tile([128, D], BF16, tag="w")
            nc.sync.dma_start(wo[:], wot_d[c * 128:(c + 1) * 128, :])
            wo_sb.append(wo)

        beta_bc = const.tile([128, H], F32, tag="beta_bc")
        nc.gpsimd.partition_broadcast(beta_bc[:], bt[:], channels=128)
        pl_bc = const.tile([128, D], F32, tag="pl_bc")
        nc.gpsimd.partition_broadcast(pl_bc[:], pl[:], channels=128)

        # kg_value.T via PE transposes (PE is otherwise idle this early)
        kgt = [kgt_p.tile([128, K], BF16, tag="kgt", name="kgt") for _ in range(DC)]
        for dchunk in range(DC):
            pt = ps.tile([128, K], BF16, tag="od", bufs=4, name="ptr")
            for c in range(KC):
                nc.tensor.transpose(
                    pt[:, c * 128:(c + 1) * 128],
                    kv_tiles[c][:, dchunk * 128:(dchunk + 1) * 128],
                    ident_bf[:])
            nc.vector.tensor_copy(kgt[dchunk][:], pt[:])

        # ---------------- phase 0: cosine-sim bias -> exp factor ----------------
        bias_all = const.tile([128, KC * H], F32, tag="bias_all")

        pl_sq = scr.tile([128, D], F32, tag="scr", name="pl_sq")
        pnorm = sm_p.tile([128, 1], F32, tag="pnorm")
        nc.scalar.activation(pl_sq[:], pl_bc[:], Act.Square, accum_out=pnorm[:])
        nc.scalar.activation(pnorm[:], pnorm[:], Act.Sqrt)
        nc.vector.tensor_scalar_max(pnorm[:], pnorm[:], EPS)
        rp_vec = const.tile([128, 1], F32, tag="rp_vec")
        nc.vector.reciprocal(rp_vec[:], pnorm[:])

        for c in range(KC):
            kk = kgk_tiles[c]
            sq = scr.tile([128, D], F32, tag="scr")
            nrm = sm_p.tile([128, 1], F32, tag="nrm")
            nc.scalar.activation(sq[:], kk[:], Act.Square, accum_out=nrm[:])
            nc.scalar.activation(nrm[:], nrm[:], Act.Sqrt)
            nc.vector.tensor_scalar_max(nrm[:], nrm[:], EPS)
            rn = sm_p.tile([128, 1], F32, tag="rn")
            nc.vector.reciprocal(rn[:], nrm[:])
            sq2 = scr.tile([128, D], F32, tag="scr")
            dot = sm_p.tile([128, 1], F32, tag="dot")
            nc.vector.scalar_tensor_tensor(
                out=sq2[:], in0=kk[:], scalar=1.0, in1=pl_bc[:],
                op0=Alu.mult, op1=Alu.mult, accum_out=dot[:])
            nc.vector.tensor_mul(dot[:], dot[:], rn[:])
            nc.vector.tensor_mul(dot[:], dot[:], rp_vec[:])
            nc.vector.tensor_scalar_mul(
                bias_all[:, c * H:(c + 1) * H], beta_bc[:], dot[:])

        # ---------------- phase 1a: k.T and v ----------------
        kt = [kt_p.tile([128, K], BF16, tag="kt", name="kt") for _ in range(DC)]
        for m in range(DC):
            pk = ps.tile([128, K], F32, tag="od", bufs=4)
            for c in range(DC):
                nc.tensor.matmul(
                    pk[:], wk_sb[c][:, m * 128:(m + 1) * 128], kgt[c][:],
                    start=(c == 0), stop=(c == DC - 1))
            nc.vector.tensor_scalar_add(kt[m][:], pk[:], bk_sb[:, m:m + 1])

        # ------- per t-window: q.T, then attention + final projection.
        # Fusing the q-projection into the window loop keeps the PE FIFO
        # from stalling on hs.T windows that are still in DMA flight. -------
        qt = [big.tile([128, T], BF16, tag="big", name="qt") for _ in range(DC)]
        ot = [big.tile([128, T], BF16, tag="big", name="ot") for _ in range(NPAIR)]

        def oproj_chain(wtb, m):
            ows = slice(wtb * TW, (wtb + 1) * TW)
            pf = ps.tile([128, TW], F32, tag="od", bufs=4, name="pf")
            for c in range(DC):
                nc.tensor.matmul(
                    pf[:], wo_sb[c][:, m * 128:(m + 1) * 128],
                    ot[c][:, ows],
                    start=(c == 0), stop=(c == DC - 1))
            fin = fin_p.tile([128, TW], F32, tag="fin")
            nc.vector.tensor_scalar_add(fin[:], pf[:], bo_sb[:, m:m + 1])
            nc.sync.dma_start(out_d[m * 128:(m + 1) * 128, ows], fin[:])

        def qproj_win(wtb):
            qws = slice(wtb * TW, (wtb + 1) * TW)
            for m in range(DC):
                pq = ps.tile([128, TW], F32, tag="od", bufs=4, name="pq")
                for c in range(DC):
                    nc.tensor.matmul(
                        pq[:], wq_sb[c][:, m * 128:(m + 1) * 128],
                        hstw[c][:, qws],
                        start=(c == 0), stop=(c == DC - 1))
                nc.vector.tensor_scalar_add(
                    qt[m][:, qws], pq[:], bq_sb[:, m:m + 1])

        qproj_win(0)
        v_sb = [v_p.tile([128, D], BF16, tag="v", name="vsb")
                for _ in range(KC)]
        for kc in range(KC):
            for n in range(2):
                pv = ps.tile([128, 384], F32, tag="od", bufs=4)
                for c in range(DC):
                    nc.tensor.matmul(
                        pv[:], kgt[c][:, kc * 128:(kc + 1) * 128],
                        wv_sb[c][:, n * 384:(n + 1) * 384],
                        start=(c == 0), stop=(c == DC - 1))
                nc.vector.tensor_copy(
                    v_sb[kc][:, n * 384:(n + 1) * 384], pv[:])


        for tb in range(NTW):
            tws = slice(tb * TW, (tb + 1) * TW)
            if tb >= 1:
                qproj_win(tb)

            for j in range(NPAIR):
                he = 2 * j
                ho = 2 * j + 1
                # previous window's final-projection chain, its matmuls
                # interleaved between the score pairs: ready PE work that
                # fills the exp-drain pacing holes
                if tb >= 1:
                    ows = slice((tb - 1) * TW, tb * TW)
                    pf = ps.tile([128, TW], F32, tag="od", bufs=4, name="pf")
                # scores (even/odd row-tiled concurrent pairs) + exp with
                # the per-partition cosine bias
                e_j = []
                for kc in range(KC):
                    h0 = kc * H + he
                    pse = ps.tile([128, TW], F32, tag="s", bufs=4)
                    nc.tensor.matmul(
                        pse[:],
                        kt[j][0:64, kc * 128:(kc + 1) * 128],
                        qt[j][0:64, tws], start=True, stop=True)
                    pso = ps.tile([128, TW], F32, tag="s", bufs=4)
                    nc.tensor.matmul(
                        pso[:],
                        kt[j][64:128, kc * 128:(kc + 1) * 128],
                        qt[j][64:128, tws], start=True, stop=True)
                    if tb >= 1:
                        for c in (2 * kc, 2 * kc + 1):
                            if c < DC:
                                nc.tensor.matmul(
                                    pf[:],
                                    wo_sb[c][:, j * 128:(j + 1) * 128],
                                    ot[c][:, ows],
                                    start=(c == 0), stop=(c == DC - 1))
                    ee = e_p.tile([128, TW], BF16, tag="e")
                    nc.scalar.activation(
                        ee[:], pse[:], Act.Exp, bias=bias_all[:, h0:h0 + 1])
                    eo = e_p.tile([128, TW], BF16, tag="e")
                    nc.scalar.activation(
                        eo[:], pso[:], Act.Exp,
                        bias=bias_all[:, h0 + 1:h0 + 2])
                    e_j.append((ee, eo))

                if tb >= 1:
                    fin = fin_p.tile([128, TW], F32, tag="fin")
                    nc.vector.tensor_scalar_add(
                        fin[:], pf[:], bo_sb[:, j:j + 1])
                    nc.sync.dma_start(
                        out_d[j * 128:(j + 1) * 128, ows], fin[:])

                # AV + denominator: po = [evenAV ; oddAV],
                # pd = [oddDen ; evenDen]; each v-matmul pairs with a
                # ones-matmul on the opposite col-group -> concurrent
                po = ps.tile([128, TW], F32, tag="od", bufs=4, name="po")
                pd = ps.tile([128, TW], F32, tag="od", bufs=4, name="pd")
                for kc in range(KC):
                    st = (kc == 0)
                    sp = (kc == KC - 1)
                    nc.tensor.matmul(
                        po[0:64, :],
                        v_sb[kc][:, he * HD:(he + 1) * HD],
                        e_j[kc][0][:], start=st, stop=sp)
                    nc.tensor.matmul(
                        pd[64:128, :], ones_bf[:, 0:64],
                        e_j[kc][0][:], start=st, stop=sp,
                        tile_position=(0, 64))
                for kc in range(KC):
                    st = (kc == 0)
                    sp = (kc == KC - 1)
                    nc.tensor.matmul(
                        po[64:128, :],
                        v_sb[kc][:, ho * HD:(ho + 1) * HD],
                        e_j[kc][1][:], start=st, stop=sp)
                    nc.tensor.matmul(
                        pd[0:64, :], ones_bf[:, 0:64],
                        e_j[kc][1][:], start=st, stop=sp,
                        tile_position=(0, 0))

                rall = r_p.tile([128, TW], F32, tag="rall", name="rall")
                nc.vector.reciprocal_approx_fast(rall[:], pd[:])
                nc.vector.tensor_mul(
                    ot[j][0:64, tws], po[0:64, :], rall[64:128, :])
                nc.vector.tensor_mul(
                    ot[j][64:128, tws], po[64:128, :], rall[0:64, :])

        for m in range(DC):
            oproj_chain(NTW - 1, m)

    nc.compile()
    return nc


def _get_program():
    if "nc" not in _CACHE:
        _CACHE["nc"] = _build_program()
    return _CACHE["nc"]


def _host_prep(inputs):
    import ml_dtypes
    bf16 = ml_dtypes.bfloat16

    f32 = lambda x: np.ascontiguousarray(np.asarray(x, dtype=np.float32))
    Wq, Wk, Wv, Wo = (f32(inputs[k]) for k in ("Wq", "Wk", "Wv", "Wo"))
    bq, bk, bv, bo = (f32(inputs[k]) for k in ("bq", "bk", "bv", "bo"))
    beta = f32(inputs["beta"])

    shared = {
        "wqt": np.ascontiguousarray((Wq.T * SCALE).astype(bf16)),
        "wkt": np.ascontiguousarray(Wk.T.astype(bf16)),
        "wvt": np.ascontiguousarray(Wv.T.astype(bf16)),
        "wot": np.ascontiguousarray(Wo.T.astype(bf16)),
        "bq": np.ascontiguousarray((bq * SCALE).reshape(DC, 128).T),
        "bk": np.ascontiguousarray(bk.reshape(DC, 128).T),
        # bv folded through Wo (sum_k softmax == 1), bo absorbed:
        "bo": np.ascontiguousarray((bo + bv @ Wo.T).reshape(DC, 128).T),
        "beta": np.ascontiguousarray(beta.reshape(1, H)),
    }

    hs = np.asarray(inputs["hidden_states"], dtype=np.float32)
    kgk = np.asarray(inputs["kg_key"], dtype=np.float32)
    kgv = np.asarray(inputs["kg_value"], dtype=np.float32)
    pooled = f32(inputs["pooled_hidden_states"])

    in_maps = []
    for b in range(BS):
        m = dict(shared)
        m["hs"] = np.ascontiguousarray(hs[b].astype(bf16))
        m["kgk"] = np.ascontiguousarray(kgk[b].astype(bf16))
        m["kgv"] = np.ascontiguousarray(kgv[b].astype(bf16))
        m["pooled"] = np.ascontiguousarray(pooled[b].reshape(1, D))
        in_maps.append(m)
    return in_maps




def _install_ntff_hook():
    """Register the axon NTFF profile hook so trace=True yields exec_time_ns.

    Only used from our own test harness (TRACE=True); the default kernel()
    path never calls this.
    """
    try:
        from antenv.axon_hooks import get_axon_ntff_profile_hook  # noqa: F401
        return
    except ImportError:
        pass
    import contextlib
    import ctypes
    import types

    so_path = "/opt/axon/libaxon_pjrt.so"
    try:
        lib = ctypes.CDLL(so_path)
    except OSError:
        return
    if not hasattr(lib, "axon_start_nrt_profile"):
        return
    lib.axon_start_nrt_profile.argtypes = [
        ctypes.POINTER(ctypes.c_int64), ctypes.c_size_t]
    lib.axon_start_nrt_profile.restype = ctypes.c_int64
    lib.axon_stop_nrt_profile.argtypes = [ctypes.c_char_p]
    lib.axon_stop_nrt_profile.restype = ctypes.c_int64

    @contextlib.contextmanager
    def _hook(output_dir, device_ids):
        import jax
        jax.devices()
        if device_ids:
            ids = (ctypes.c_int64 * len(device_ids))(*device_ids)
            rc = lib.axon_start_nrt_profile(ids, len(device_ids))
        else:
            rc = lib.axon_start_nrt_profile(None, 0)
        if rc != 0:
            raise RuntimeError(f"axon_start_nrt_profile rc={rc}")
        try:
            yield
        finally:
            n = lib.axon_stop_nrt_profile(str(output_dir).encode())
            print(f"profile: {n} file(s) written to {output_dir}",
                  file=sys.stderr)

    mod = types.ModuleType("antenv.axon_hooks")
    mod.get_axon_ntff_profile_hook = lambda: _hook
    mod.set_axon_ntff_profile_hook = lambda h: None
    sys.modules["antenv.axon_hooks"] = mod


def kernel(**inputs):
    global LAST_EXEC_NS
    _ensure_path()
    from concourse import bass_utils

    if TRACE:
        _install_ntff_hook()
    nc = _get_program()
    in_maps = _host_prep(inputs)
    res = bass_utils.run_bass_kernel_spmd(
        nc, in_maps, core_ids=list(range(BS)), trace=TRACE)
    LAST_EXEC_NS = res.exec_time_ns
    # device output is out.T [D, T]; un-transpose per example
    out = np.stack([res.results[b]["out"].T for b in range(BS)], axis=0)
    return np.ascontiguousarray(out).astype(np.float32)
